# revision 8
# baseline (speedup 1.0000x reference)
"""Trainium2 Bass kernel for nn_Microscope (PSF scatter-add).

Sharding: 8 cores = (b in 0..4) x (h-half in {0,1}).  Each core owns output
rows (b, h_half*128 .. +128) and processes every emitter whose patch rows
intersect its 128-row slab (boundary emitters are duplicated to both
h-halves; each core only writes its own rows, so the output is an exact
partition -- no collectives).

Per core (data-specialized program, compiled at call time):
 - emitters sorted by w, packed 4 per "group" into a [128, 448] bf16 staging
   tile (emitter g at partitions 32g..32g+21; pad rows/cols memset to 0 once).
 - ACT: relu + accum_out row-sums.
 - PE+DVE: per 8-group batch, indicator matmuls + reciprocal produce the
   per-emitter scale (i_val * 1e6 / sum) broadcast to [128, 1] slots.
 - DVE: tensor_scalar multiply -> scaled bf16 patches.
 - PE: per-emitter row-routing matmuls.  lhsT = 128-col slice of a static
   block-diagonal shift matrix Z_g (row 32g+k routes to out row k+base; rows
   outside the slab fall outside the slice window = dropped).  rhs = patch
   columns.  out = PSUM, a 7-bank ring over w (bank = 4 w x 128 d),
   accumulating all emitters' contributions per 4-w tile.
 - ACT: evacuate finished psum tiles (crop d) -> SBUF -> DMA to DRAM output.
"""

import threading
from contextlib import ExitStack

import ml_dtypes
import numpy as np

import concourse.bass as bass
LAST = None
import concourse.tile as tile
from concourse import bacc, mybir
from concourse import bass_utils

BF16 = mybir.dt.bfloat16
F32 = mybir.dt.float32
AF = mybir.ActivationFunctionType
BF16NP = ml_dtypes.bfloat16

BS, CH, H, W, D = 4, 1, 256, 256, 64
PH, PW, PD = 21, 21, 21
SCALE_MULT = 10000.0 * 100.0  # folded into i_val
HALF = 128          # h rows per core
GB = 8              # groups per normalization batch
NTILES = 70         # 4-w psum tiles covering w_abs in [-12, 268)
NBANKS = 7          # ring size (8th bank for the normalization chain)
PATCH_COLS = PW * PD  # 441


def _host_pack(psf_raw, i_val, b, h, w, d):
    cores = []
    for core in range(8):
        b_t, half = core >> 1, core & 1
        lo = half * HALF
        sel = np.where(
            (b == b_t) & (h - PH // 2 <= lo + HALF - 1) & (h + PH // 2 >= lo)
        )[0]
        order = np.argsort(w[sel], kind="stable")
        idx = sel[order]
        ne = len(idx)
        if ne == 0:
            cores.append(None)
            continue
        npad = (-ne) % 4
        if npad:
            idx = np.concatenate([idx, np.repeat(idx[:1], npad)])
        ival = i_val[idx].astype(np.float32) * SCALE_MULT
        if npad:
            ival[ne:] = 0.0
        ng = len(idx) // 4
        nb = (ng + GB - 1) // GB
        psf_packed = np.ascontiguousarray(
            psf_raw[idx].reshape(ng, 4, PH, PATCH_COLS).astype(BF16NP))
        ival_p = np.zeros((nb, 4, GB), np.float32)
        iv = ival.reshape(ng, 4)  # [group, slot]
        for g in range(ng):
            ival_p[g // GB, :, g % GB] = iv[g]
        he, we, de = h[idx], w[idx], d[idx]
        base = he.astype(np.int64) - PH // 2 - lo          # in [-20, 127]
        c0 = (127 - base).astype(np.int64)                 # in [0, 147]
        d0 = de.astype(np.int64) + 2                       # in [2, 66)
        t0 = (we.astype(np.int64) + 2) // 4                # first psum tile
        cores.append(dict(ne=ne, ntot=len(idx), ng=ng, nb=nb,
                          psf=psf_packed, ival=ival_p,
                          c0=c0, d0=d0, w=we.astype(np.int64), t0=t0))
    return cores


def _consts():
    p = np.arange(128)[:, None]
    c = np.arange(288)[None, :]
    zconst = np.stack([
        (((c - (p % 32)) == 127) & (p // 32 == g)).astype(BF16NP)
        for g in range(4)])                                    # [4, 128, 288]
    ind = (np.arange(128)[:, None] // 32 == np.arange(4)[None, :]).astype(np.float32)
    indT = np.ascontiguousarray(ind.T)
    return zconst, ind, indT


def _build_program(cd):
    ng, nb, ntot, ne = cd["ng"], cd["nb"], cd["ntot"], cd["ne"]
    nc = bacc.Bacc("TRN2", target_bir_lowering=False, debug=False)
    psf_d = nc.dram_tensor("psf", [ng, 4, PH, PATCH_COLS], BF16,
                           kind="ExternalInput").ap()
    ival_d = nc.dram_tensor("ival", [nb, 4, GB], F32, kind="ExternalInput").ap()
    z_d = nc.dram_tensor("zconst", [4, 128, 288], BF16, kind="ExternalInput").ap()
    ind_d = nc.dram_tensor("ind", [128, 4], F32, kind="ExternalInput").ap()
    indT_d = nc.dram_tensor("indT", [4, 128], F32, kind="ExternalInput").ap()
    out_d = nc.dram_tensor("out", [HALF, W, D], F32, kind="ExternalOutput").ap()

    STG_BUFS = 16
    SCL_BUFS = 8

    with tile.TileContext(nc) as tc:
        with ExitStack() as ctx:
            const = ctx.enter_context(tc.tile_pool(name="const", bufs=1))
            stgp = ctx.enter_context(tc.tile_pool(name="stgp", bufs=STG_BUFS))
            sclp = ctx.enter_context(tc.tile_pool(name="sclp", bufs=SCL_BUFS))
            normp = ctx.enter_context(tc.tile_pool(name="normp", bufs=2))
            evp = ctx.enter_context(tc.tile_pool(name="evp", bufs=4))
            psum = ctx.enter_context(tc.tile_pool(name="psum", bufs=1, space="PSUM"))

            z_t = const.tile([128, 4 * 288], BF16)
            for g in range(4):
                nc.gpsimd.dma_start(z_t[:, 288 * g:288 * (g + 1)], z_d[g])
            ind_t = const.tile([128, 4], F32)
            nc.gpsimd.dma_start(ind_t[:], ind_d[:])
            indT_t = const.tile([4, 128], F32)
            nc.gpsimd.dma_start(indT_t[:], indT_d[:])
            zl = const.tile([1, 128], BF16, tag="zl")
            nc.gpsimd.memset(zl[:], 0.0)
            zr = const.tile([1, 512], BF16, tag="zr")
            nc.gpsimd.memset(zr[:], 0.0)

            stg_tiles = [const.tile([128, 448], BF16, tag=f"stg{i}", name=f"stg{i}")
                         for i in range(STG_BUFS)]
            rlu_tiles = [const.tile([128, 448], BF16, tag=f"rlu{i}", name=f"rlu{i}")
                         for i in range(STG_BUFS)]
            for t in stg_tiles:
                nc.vector.memset(t[:], 0.0)
            scl_tiles = [const.tile([128, 448], BF16, tag=f"scl{i}", name=f"scl{i}")
                         for i in range(SCL_BUFS)]

            ring = psum.tile([128, NBANKS * 512], F32)
            ring_r = ring[:].rearrange("p (w d) -> p w d", d=128)
            ps_norm = psum.tile([128, 512], F32)

            rows_t = [const.tile([128, GB], F32, tag=f"rows{i}", name=f"rows{i}") for i in range(2)]
            scale_t = [const.tile([128, GB], F32, tag=f"scale{i}", name=f"scale{i}") for i in range(2)]
            recip_t = [const.tile([4, GB], F32, tag=f"recip{i}", name=f"recip{i}") for i in range(2)]
            ival_t = [const.tile([4, GB], F32, tag=f"ivalt{i}", name=f"ivalt{i}") for i in range(2)]
            for t in rows_t:
                nc.vector.memset(t[:], 1.0)

            def zero_tile(t):
                r = t % NBANKS
                nc.tensor.matmul(ring[:, 512 * r:512 * (r + 1)], zl[:], zr[:],
                                 start=True, stop=False, skip_group_check=True)

            def evac_tile(t):
                if not (3 <= t <= 66):
                    return
                ev = evp.tile([128, 4 * D], F32, tag="ev", name="ev")
                r = t % NBANKS
                nc.scalar.activation(
                    ev[:].rearrange("p (w d) -> p w d", d=D),
                    ring_r[:, 4 * r:4 * r + 4, 12:76], AF.Copy)
                wb = 4 * (t - 3)
                nc.scalar.dma_start(out_d[:, wb:wb + 4, :], ev[:])

            stg_map = {}
            scl_map = {}
            next_load = 0
            next_chain = 0

            def load_group(g):
                nonlocal next_load
                assert g == next_load
                st = stg_tiles[g % STG_BUFS]
                for slot in range(4):
                    nc.sync.dma_start(st[32 * slot:32 * slot + PH, 0:PATCH_COLS],
                                      psf_d[g, slot])
                rt = rows_t[(g // GB) % 2]
                rl = rlu_tiles[g % STG_BUFS]
                nc.scalar.activation(rl[:], st[:], AF.Relu,
                                     accum_out=rt[:, (g % GB):(g % GB) + 1])
                stg_map[g] = rl
                next_load += 1

            def norm_chain(k):
                nonlocal next_chain
                assert k == next_chain
                rt, sct, rct, ivt = (rows_t[k % 2], scale_t[k % 2],
                                     recip_t[k % 2], ival_t[k % 2])
                nc.gpsimd.dma_start(ivt[:], ival_d[k])
                nc.tensor.matmul(ps_norm[0:4, 0:GB], ind_t[:], rt[:],
                                 start=True, stop=True, skip_group_check=True)
                nc.vector.reciprocal(rct[:], ps_norm[0:4, 0:GB])
                nc.vector.tensor_mul(rct[:], rct[:], ivt[:])
                nc.tensor.matmul(ps_norm[0:128, 64:64 + GB], indT_t[:], rct[:],
                                 start=True, stop=True, skip_group_check=True)
                nc.vector.tensor_copy(sct[:], ps_norm[0:128, 64:64 + GB])
                next_chain += 1

            def ensure_scaled(g):
                """Load groups through g's batch end, run the chain, scale g."""
                k = g // GB
                batch_end = min(ng - 1, k * GB + GB - 1)
                while next_load <= batch_end:
                    load_group(next_load)
                while next_chain <= k:
                    norm_chain(next_chain)
                if g not in scl_map:
                    sc = scl_tiles[g % SCL_BUFS]
                    nc.vector.tensor_scalar(
                        sc[:], stg_map[g][:],
                        scale_t[k % 2][:, (g % GB):(g % GB) + 1],
                        None, mybir.AluOpType.mult)
                    scl_map[g] = sc
                return scl_map[g]

            def emit_emitter_mms(e, sc):
                g_slot = e % 4
                c0, d0, we, t0 = (int(cd["c0"][e]), int(cd["d0"][e]),
                                  int(cd["w"][e]), int(cd["t0"][e]))
                lhsT = z_t[:, 288 * g_slot + c0: 288 * g_slot + c0 + 128]
                for t in range(t0, t0 + 6):
                    j0 = max(0, 4 * t - 12 - (we - 10))
                    j1 = min(PW, 4 * t - 8 - (we - 10))
                    nj = j1 - j0
                    if nj <= 0:
                        continue
                    wl = (we - 10 + j0) - (4 * t - 12)
                    r = t % NBANKS
                    rhs = sc[:, j0 * PD:(j0 + nj) * PD].rearrange(
                        "p (j d) -> p j d", d=PD)
                    out = ring_r[:, 4 * r + wl:4 * r + wl + nj, d0:d0 + PD]
                    nc.tensor.matmul(out, lhsT, rhs, start=False, stop=False,
                                     skip_group_check=True)

            # ---- main schedule ----
            t0s = cd["t0"]
            step = 0
            for t in range(min(6, NTILES)):
                zero_tile(t)
            for e in range(ntot):
                if e < ne:
                    s = int(t0s[e])
                    while step < s:
                        evac_tile(step)
                        step += 1
                        if step + 5 < NTILES:
                            zero_tile(step + 5)
                    sc = ensure_scaled(e // 4)
                    emit_emitter_mms(e, sc)
                else:
                    g = e // 4
                    if g == next_load:
                        load_group(g)
            while next_load < ng:
                load_group(next_load)
            while next_chain < nb:
                norm_chain(next_chain)
            while step < NTILES:
                evac_tile(step)
                step += 1
                if step + 5 < NTILES:
                    zero_tile(step + 5)

    nc.compile()
    return nc


def kernel(psf_raw, i_val, b, c, h, w, d):
    psf_raw = np.asarray(psf_raw)
    i_val = np.asarray(i_val)
    b = np.asarray(b); h = np.asarray(h); w = np.asarray(w); d = np.asarray(d)
    n = psf_raw.shape[0]
    psf_flat = psf_raw.reshape(n, PH, PW, PD)

    cores = _host_pack(psf_flat, i_val, b, h, w, d)
    zconst, ind, indT = _consts()

    ncs = [None] * 8
    errs = []

    def build(i):
        try:
            if cores[i] is not None:
                ncs[i] = _build_program(cores[i])
        except BaseException as exc:
            errs.append((i, exc))
            raise

    threads = [threading.Thread(target=build, args=(i,)) for i in range(8)]
    for t in threads:
        t.start()
    for t in threads:
        t.join()
    if errs:
        raise errs[0][1]

    import jax
    devices = jax.devices()
    results = [None] * 8

    def run(i):
        if ncs[i] is None:
            results[i] = {"out": np.zeros((HALF, W, D), np.float32)}
            return
        cd = cores[i]
        in_map = {
            "psf": cd["psf"], "ival": cd["ival"],
            "zconst": zconst, "ind": ind, "indT": indT,
        }
        try:
            with jax.default_device(devices[i]):
                res = bass_utils.run_bass_kernel_spmd(ncs[i], [in_map], core_ids=[0])
            results[i] = res.results[0]
        except BaseException as exc:
            errs.append((i, exc))
            raise

    rthreads = [threading.Thread(target=run, args=(i,)) for i in range(8)]
    for t in rthreads:
        t.start()
    for t in rthreads:
        t.join()
    if errs:
        raise errs[0][1]

    global LAST
    LAST = {"cores": cores, "ncs": ncs, "zconst": zconst, "ind": ind, "indT": indT}

    out = np.zeros((BS, CH, H, W, D), np.float32)
    for core in range(8):
        b_t, half = core >> 1, core & 1
        out[b_t, 0, half * HALF:(half + 1) * HALF] = results[core]["out"]
    return out


# revision 10
# speedup vs baseline: 2.7101x; 2.7101x over previous
"""Trainium2 Bass kernel for nn_Microscope (PSF scatter-add).

Sharding: 8 cores = (b in 0..4) x (h-half in {0,1}).  Each core owns output
rows (b, h_half*128 .. +128) and processes every emitter whose patch rows
intersect its 128-row slab (boundary emitters are duplicated to both
h-halves; each core only writes its own rows, so the output is an exact
partition -- no collectives).

Per core (data-specialized program, compiled at call time):
 - emitters sorted by w, packed 6 per "group" into a [128, 448] bf16 staging
   window (emitter s at partitions 21s..21s+21); 4 group-images per load DMA.
 - ACT: relu + accum_out row-sums.
 - PE+DVE: per 8-group batch, indicator matmuls + reciprocal produce the
   per-emitter scale (i_val * 1e6 / sum) broadcast to [128, 1] slots.
 - DVE: tensor_scalar multiply -> scaled bf16 patches.
 - PE: per-emitter row-routing matmuls.  lhsT = 128-col slice of a static
   block-diagonal shift matrix Z_s (row 21s+k routes to out row k+base; rows
   outside the slab fall outside the slice window = dropped).  rhs = patch
   columns.  out = PSUM, a 7-bank ring over w (bank = 4 w x 128 d),
   accumulating all emitters' contributions per 4-w tile.
 - ACT: evacuate finished psum tiles (crop d) -> SBUF -> DMA to DRAM output.
"""

import threading
from contextlib import ExitStack

import ml_dtypes
import numpy as np

import concourse.bass as bass
import concourse.tile as tile
from concourse import bacc, mybir
from concourse import bass_utils

LAST = None
BF16 = mybir.dt.bfloat16
F32 = mybir.dt.float32
AF = mybir.ActivationFunctionType
BF16NP = ml_dtypes.bfloat16

BS, CH, H, W, D = 4, 1, 256, 256, 64
PH, PW, PD = 21, 21, 21
SCALE_MULT = 10000.0 * 100.0  # folded into i_val
HALF = 128          # h rows per core
G = 6               # emitters per staging group (6*21 = 126 partitions)
GB = 8              # groups per normalization batch
LB = 4              # groups per load DMA
NW = 16             # staging windows
NTILES = 70         # 4-w psum tiles covering w_abs in [-12, 268)
NBANKS = 7          # ring size (8th bank for the normalization chain)
PATCH_COLS = PW * PD  # 441
WIN = 448           # staging window width


def _host_pack(psf_raw, i_val, b, h, w, d):
    cores = []
    for core in range(8):
        b_t, half = core >> 1, core & 1
        lo = half * HALF
        sel = np.where(
            (b == b_t) & (h - PH // 2 <= lo + HALF - 1) & (h + PH // 2 >= lo)
        )[0]
        order = np.argsort(w[sel], kind="stable")
        idx = sel[order]
        ne = len(idx)
        if ne == 0:
            cores.append(None)
            continue
        npad = (-ne) % G
        if npad:
            idx = np.concatenate([idx, np.repeat(idx[:1], npad)])
        ival = i_val[idx].astype(np.float32) * SCALE_MULT
        if npad:
            ival[ne:] = 0.0
        ntot = len(idx)
        ng = ntot // G
        nb = (ng + GB - 1) // GB
        nb4 = (ng + LB - 1) // LB
        # psf packed partition-major per load-batch: [nb4, 126, LB, 441]
        pf = psf_raw[idx].reshape(ng, G * PH, PATCH_COLS).astype(BF16NP)
        psf_packed = np.zeros((nb4, G * PH, LB, PATCH_COLS), BF16NP)
        for g in range(ng):
            psf_packed[g // LB, :, g % LB, :] = pf[g]
        ival_p = np.zeros((nb, G, GB), np.float32)
        iv = ival.reshape(ng, G)  # [group, slot]
        for g in range(ng):
            ival_p[g // GB, :, g % GB] = iv[g]
        he, we, de = h[idx], w[idx], d[idx]
        base = he.astype(np.int64) - PH // 2 - lo          # in [-20, 127]
        c0 = (127 - base).astype(np.int64)                 # in [0, 147]
        d0 = de.astype(np.int64) + 2                       # in [2, 66)
        t0 = (we.astype(np.int64) + 2) // 4                # first psum tile
        cores.append(dict(ne=ne, ntot=ntot, ng=ng, nb=nb, nb4=nb4,
                          psf=psf_packed, ival=ival_p,
                          c0=c0, d0=d0, w=we.astype(np.int64), t0=t0))
    return cores


def _consts():
    p = np.arange(128)[:, None]
    c = np.arange(288)[None, :]
    zconst = np.stack([
        (((c - (p - 21 * s)) == 127) & (p // 21 == s) & (p < 126)).astype(BF16NP)
        for s in range(G)])                                 # [6, 128, 288]
    ind = ((np.arange(128)[:, None] // 21 == np.arange(G)[None, :])
           & (np.arange(128)[:, None] < 126)).astype(np.float32)   # [128, 6]
    indT = np.ascontiguousarray(ind.T)                      # [6, 128]
    return zconst, ind, indT


def _build_program(cd):
    ng, nb, nb4, ntot, ne = cd["ng"], cd["nb"], cd["nb4"], cd["ntot"], cd["ne"]
    nc = bacc.Bacc("TRN2", target_bir_lowering=False, debug=False)
    psf_d = nc.dram_tensor("psf", [nb4, G * PH, LB, PATCH_COLS], BF16,
                           kind="ExternalInput").ap()
    ival_d = nc.dram_tensor("ival", [nb, G, GB], F32, kind="ExternalInput").ap()
    z_d = nc.dram_tensor("zconst", [G, 128, 288], BF16, kind="ExternalInput").ap()
    ind_d = nc.dram_tensor("ind", [128, G], F32, kind="ExternalInput").ap()
    indT_d = nc.dram_tensor("indT", [G, 128], F32, kind="ExternalInput").ap()
    out_d = nc.dram_tensor("out", [HALF, W, D], F32, kind="ExternalOutput").ap()

    with tile.TileContext(nc) as tc:
        with ExitStack() as ctx:
            const = ctx.enter_context(tc.tile_pool(name="const", bufs=1))
            evp = ctx.enter_context(tc.tile_pool(name="evp", bufs=4))
            psum = ctx.enter_context(tc.tile_pool(name="psum", bufs=1, space="PSUM"))

            z_t = const.tile([128, G * 288], BF16)
            for s in range(G):
                nc.gpsimd.dma_start(z_t[:, 288 * s:288 * (s + 1)], z_d[s])
            ind_t = const.tile([128, G], F32)
            nc.gpsimd.dma_start(ind_t[:], ind_d[:])
            indT_t = const.tile([G, 128], F32)
            nc.gpsimd.dma_start(indT_t[:], indT_d[:])
            zl = const.tile([1, 128], BF16, tag="zl")
            nc.gpsimd.memset(zl[:], 0.0)
            zr = const.tile([1, 512], BF16, tag="zr")
            nc.gpsimd.memset(zr[:], 0.0)

            stg = const.tile([128, NW * WIN], BF16)
            nc.vector.memset(stg[:], 0.0)
            rlu = const.tile([128, NW * WIN], BF16)
            scl = const.tile([128, NW * WIN], BF16)

            ring = psum.tile([128, NBANKS * 512], F32)
            ring_r = ring[:].rearrange("p (w d) -> p w d", d=128)
            ps_norm = psum.tile([128, 512], F32)

            rows_t = [const.tile([128, GB], F32, tag=f"rows{i}", name=f"rows{i}")
                      for i in range(2)]
            scale_t = [const.tile([128, GB], F32, tag=f"scale{i}", name=f"scale{i}")
                       for i in range(2)]
            recip_t = [const.tile([G, GB], F32, tag=f"recip{i}", name=f"recip{i}")
                       for i in range(2)]
            ival_t = [const.tile([G, GB], F32, tag=f"ivalt{i}", name=f"ivalt{i}")
                      for i in range(2)]

            def zero_tile(t):
                r = t % NBANKS
                nc.tensor.matmul(ring[:, 512 * r:512 * (r + 1)], zl[:], zr[:],
                                 start=True, stop=False, skip_group_check=True)

            def evac_tile(t):
                if not (3 <= t <= 66):
                    return
                ev = evp.tile([128, 4 * D], F32, tag="ev", name="ev")
                r = t % NBANKS
                nc.scalar.activation(
                    ev[:].rearrange("p (w d) -> p w d", d=D),
                    ring_r[:, 4 * r:4 * r + 4, 12:76], AF.Copy)
                wb = 4 * (t - 3)
                nc.scalar.dma_start(out_d[:, wb:wb + 4, :], ev[:])

            next_dma = 0       # next load-batch (LB groups) to DMA
            next_relu = 0      # next group to relu
            next_chain = 0     # next norm batch
            scl_done = set()

            def win(tile_, g):
                return tile_[:, WIN * (g % NW):WIN * (g % NW) + WIN]

            def dma_batch():
                nonlocal next_dma
                bi = next_dma
                g0 = bi * LB
                ngrp = min(LB, ng - g0)
                w0 = g0 % NW
                assert w0 + ngrp <= NW
                dst = stg[0:G * PH, WIN * w0:WIN * (w0 + ngrp)].rearrange(
                    "p (g c) -> p g c", c=WIN)[:, :, 0:PATCH_COLS]
                src = psf_d[bi, :, 0:ngrp, :]
                nc.sync.dma_start(dst, src)
                next_dma += 1

            def relu_group(g):
                nonlocal next_relu
                assert g == next_relu
                while next_dma < nb4 and next_dma * LB <= g + 2 * LB:
                    dma_batch()
                rt = rows_t[(g // GB) % 2]
                nc.scalar.activation(win(rlu, g), win(stg, g), AF.Relu,
                                     accum_out=rt[:, (g % GB):(g % GB) + 1])
                next_relu += 1

            def norm_chain(k):
                nonlocal next_chain
                assert k == next_chain
                rt, sct, rct, ivt = (rows_t[k % 2], scale_t[k % 2],
                                     recip_t[k % 2], ival_t[k % 2])
                nc.gpsimd.dma_start(ivt[:], ival_d[k])
                nc.tensor.matmul(ps_norm[0:G, 0:GB], ind_t[:], rt[:],
                                 start=True, stop=True, skip_group_check=True)
                nc.vector.reciprocal(rct[:], ps_norm[0:G, 0:GB])
                nc.vector.tensor_mul(rct[:], rct[:], ivt[:])
                nc.tensor.matmul(ps_norm[0:128, 64:64 + GB], indT_t[:], rct[:],
                                 start=True, stop=True, skip_group_check=True)
                nc.vector.tensor_copy(sct[:], ps_norm[0:128, 64:64 + GB])
                next_chain += 1

            def ensure_scaled(g):
                k = g // GB
                batch_end = min(ng - 1, k * GB + GB - 1)
                while next_relu <= batch_end:
                    relu_group(next_relu)
                while next_chain <= k:
                    norm_chain(next_chain)
                if g not in scl_done:
                    nc.vector.tensor_scalar(
                        win(scl, g), win(rlu, g),
                        scale_t[k % 2][:, (g % GB):(g % GB) + 1],
                        None, mybir.AluOpType.mult)
                    scl_done.add(g)
                return win(scl, g)

            def emit_emitter_mms(e, sc):
                s = e % G
                c0, d0, we, t0 = (int(cd["c0"][e]), int(cd["d0"][e]),
                                  int(cd["w"][e]), int(cd["t0"][e]))
                lhsT = z_t[:, 288 * s + c0: 288 * s + c0 + 128]
                for t in range(t0, t0 + 6):
                    j0 = max(0, 4 * t - 12 - (we - 10))
                    j1 = min(PW, 4 * t - 8 - (we - 10))
                    nj = j1 - j0
                    if nj <= 0:
                        continue
                    wl = (we - 10 + j0) - (4 * t - 12)
                    r = t % NBANKS
                    rhs = sc[:, j0 * PD:(j0 + nj) * PD].rearrange(
                        "p (j d) -> p j d", d=PD)
                    out = ring_r[:, 4 * r + wl:4 * r + wl + nj, d0:d0 + PD]
                    nc.tensor.matmul(out, lhsT, rhs, start=False, stop=False,
                                     skip_group_check=True)

            # ---- main schedule ----
            t0s = cd["t0"]
            step = 0
            for t in range(min(6, NTILES)):
                zero_tile(t)
            for e in range(ntot):
                if e >= ne:
                    continue
                s = int(t0s[e])
                while step < s:
                    evac_tile(step)
                    step += 1
                    if step + 5 < NTILES:
                        zero_tile(step + 5)
                sc = ensure_scaled(e // G)
                emit_emitter_mms(e, sc)
            while step < NTILES:
                evac_tile(step)
                step += 1
                if step + 5 < NTILES:
                    zero_tile(step + 5)

    nc.compile()
    return nc


def kernel(psf_raw, i_val, b, c, h, w, d):
    psf_raw = np.asarray(psf_raw)
    i_val = np.asarray(i_val)
    b = np.asarray(b); h = np.asarray(h); w = np.asarray(w); d = np.asarray(d)
    n = psf_raw.shape[0]
    psf_flat = psf_raw.reshape(n, PH, PW, PD)

    cores = _host_pack(psf_flat, i_val, b, h, w, d)
    zconst, ind, indT = _consts()

    ncs = [None] * 8
    errs = []

    def build(i):
        try:
            if cores[i] is not None:
                ncs[i] = _build_program(cores[i])
        except BaseException as exc:
            errs.append((i, exc))
            raise

    threads = [threading.Thread(target=build, args=(i,)) for i in range(8)]
    for t in threads:
        t.start()
    for t in threads:
        t.join()
    if errs:
        raise errs[0][1]

    import jax
    devices = jax.devices()
    results = [None] * 8

    def run(i):
        if ncs[i] is None:
            results[i] = {"out": np.zeros((HALF, W, D), np.float32)}
            return
        cd = cores[i]
        in_map = {
            "psf": cd["psf"], "ival": cd["ival"],
            "zconst": zconst, "ind": ind, "indT": indT,
        }
        try:
            with jax.default_device(devices[i]):
                res = bass_utils.run_bass_kernel_spmd(ncs[i], [in_map], core_ids=[0])
            results[i] = res.results[0]
        except BaseException as exc:
            errs.append((i, exc))
            raise

    rthreads = [threading.Thread(target=run, args=(i,)) for i in range(8)]
    for t in rthreads:
        t.start()
    for t in rthreads:
        t.join()
    if errs:
        raise errs[0][1]

    global LAST
    LAST = {"cores": cores, "ncs": ncs, "zconst": zconst, "ind": ind, "indT": indT}

    out = np.zeros((BS, CH, H, W, D), np.float32)
    for core in range(8):
        b_t, half = core >> 1, core & 1
        out[b_t, 0, half * HALF:(half + 1) * HALF] = results[core]["out"]
    return out
